# revision 1
# baseline (speedup 1.0000x reference)
"""Trainium2 Bass kernel for CapLayer2 (1x1-conv capsule layer with dynamic routing).

Sharding: data-parallel over batch — 8 batches per core on 8 NeuronCores.

Per-core design (2 waves x 4 batches):
  - The 1x1 conv produces BOTH pred layouts on TensorE in float32r:
      predT [i-part, o]  (for the s matmuls, contraction over i=1024)
      pred  [o-part, i]  (for the delta matmuls, contraction over o=320)
    The conv bias is folded into the evictions: a DVE tensor-add against a
    partition-broadcast bias tile for predT, and the per-partition bias
    operand of the ScalarE activation for pred.
  - Routing state b/c lives in [i-part, (batch, itile, j)] layout so the
    softmax over j (J=10) is a free-dim grouped reduction; softmax runs
    per batch so each batch's s matmuls start as soon as its own delta
    transposes land (batch-level pipelining).
  - s/delta matmuls use per-batch [10, N] PSUM tiles at base partition 0
    (column tiling is illegal for 4-byte dtypes), so squash norms are
    natural per-partition accumulators (Square with accum_out).
  - sqrt is computed as exp(0.5*ln) and get_activation_tables is pinned to
    natural_log_exp_and_others so the ACT engine never reloads its table.
  - delta [10, 1024] rows are PE-transposed back to [i-part, j] in 128-col
    blocks packed into one PSUM tile, giving a single [128, 80] DVE add
    into b per batch-iteration.
"""

import numpy as np
from contextlib import ExitStack

import concourse.bacc as bacc
import concourse.bass as bass
import concourse.hw_specs as hw_specs

# Force every activation onto the one table that contains all functions this
# kernel uses (Copy/Identity/Exp/Ln/Square) so the ACT engine loads its
# function table exactly once instead of thrashing between sets.
_ONE_TABLE = "natural_log_exp_and_others"
_orig_get_tables = hw_specs.get_activation_tables


def _pinned_tables(arch):
    tabs = _orig_get_tables(arch)
    return {k: (v if k == _ONE_TABLE else set()) for k, v in tabs.items()}


bacc.get_activation_tables = _pinned_tables
import concourse.tile as tile
from concourse import mybir
from concourse.bass_utils import run_bass_kernel_spmd

F32 = mybir.dt.float32
F32R = mybir.dt.float32r
AF = mybir.ActivationFunctionType
OP = mybir.AluOpType

N_CORES = 8
BS = 64
C_IN = 256
J = 10
D = 32
O = J * D          # 320
I = 1024           # 32*32 pixels
ROUTE_NUM = 3
B_PER_CORE = BS // N_CORES   # 8
WAVE = 4
N_WAVES = B_PER_CORE // WAVE
N_IT = I // 128    # 8
N_KT = C_IN // 128 # 2
N_OT = 3           # o tiles: 128, 128, 64


def r(ap):
    return ap.bitcast(F32R)


def strip_gather(t, kw=128):
    """[kw, 128] tile -> [kw, WAVE, J] AP selecting cols 32*b+j."""
    return bass.AP(tensor=t.tensor, offset=t.offset, ap=[list(t.ap[0]), [32, WAVE], [1, J]])[:kw]


def build_kernel(stage=5):
    nc = bacc.Bacc("TRN2", target_bir_lowering=False, debug=False, num_devices=1)

    x_d = nc.dram_tensor("x", [B_PER_CORE, C_IN, I], F32R, kind="ExternalInput")
    wt_d = nc.dram_tensor("wt", [C_IN, O], F32R, kind="ExternalInput")   # W.T
    wb_d = nc.dram_tensor("wb", [1, O], F32R, kind="ExternalInput")
    out_d = nc.dram_tensor("v", [B_PER_CORE, J, D], F32, kind="ExternalOutput")

    ident_np = np.eye(128, dtype=np.float32)
    bm = np.zeros((128, O), dtype=np.float32)
    for b4 in range(WAVE):
        for j in range(J):
            bm[32 * b4 + j, 32 * j:32 * j + 32] = 1.0
    ident_d = nc.inline_tensor(ident_np, name="ident")
    bmask_d = nc.inline_tensor(bm, name="bmask")
    c0_d = nc.inline_tensor(np.full((128, J), 1.0 / J, dtype=np.float32), name="c0")

    with tile.TileContext(nc) as tc:
        with ExitStack() as ctx:
            consts = ctx.enter_context(tc.tile_pool(name="consts", bufs=1))
            xpool = ctx.enter_context(tc.tile_pool(name="xp", bufs=3))
            ppool = ctx.enter_context(tc.tile_pool(name="pp", bufs=WAVE + 2))
            state = ctx.enter_context(tc.tile_pool(name="st", bufs=2))
            ps_conv = ctx.enter_context(tc.tile_pool(name="psc", bufs=2, space="PSUM"))
            ps_st = ctx.enter_context(tc.tile_pool(name="pss", bufs=1, space="PSUM"))
            ps_tp = ctx.enter_context(tc.tile_pool(name="pst", bufs=3, space="PSUM"))
            ps_dp = ctx.enter_context(tc.tile_pool(name="psd", bufs=2, space="PSUM"))

            # ---- constants ----
            wt_sb = consts.tile([128, N_KT * O], F32R)
            nc.sync.dma_start(
                out=wt_sb.rearrange("p (k o) -> p k o", o=O),
                in_=wt_d.ap().rearrange("(k p) o -> p k o", p=128),
            )
            bias_b128 = consts.tile([128, O], F32)
            wb_bc = bass.AP(
                tensor=wb_d, offset=0, ap=[[0, 128], [1, O]]
            ).bitcast(F32)
            nc.sync.dma_start(out=bias_b128, in_=wb_bc)
            routing_consts = {}

            def load_routing_consts():
                # Emitted after the first batches' x DMAs are enqueued so the
                # small/scatter transfers don't delay the startup-critical x.
                bias_col = consts.tile([128, N_OT], F32)
                for m in range(N_OT):
                    mw = 128 if m < 2 else 64
                    nc.sync.dma_start(
                        out=bias_col[0:mw, m:m + 1],
                        in_=wb_d.ap().bitcast(F32)[0:1, 128 * m:128 * m + mw],
                    )
                ident_sb = consts.tile([128, 16], F32)
                nc.sync.dma_start(out=ident_sb, in_=ident_d.ap()[:, :16])
                bmask_sb = consts.tile([128, O], F32)
                nc.sync.dma_start(out=bmask_sb, in_=bmask_d.ap())
                c0_sb = consts.tile([128, J], F32R)
                nc.sync.dma_start(out=c0_sb, in_=r(c0_d.ap()))
                routing_consts.update(
                    bias_col=bias_col, ident_sb=ident_sb, bmask_sb=bmask_sb,
                    c0_sb=c0_sb,
                )

            for wave in range(N_WAVES):
                # ======== conv: both layouts, 4 batches ========
                predT, pred = [], []
                for b in range(WAVE):
                    gb = wave * WAVE + b
                    x_sb = xpool.tile([128, N_KT * I], F32R, tag="x")
                    for k in range(N_KT):
                        nc.sync.dma_start(
                            out=x_sb[:, k * I:(k + 1) * I],
                            in_=x_d.ap()[gb][k * 128:(k + 1) * 128, :],
                        )
                    if wave == 0 and b == 0:
                        load_routing_consts()
                    bias_col = routing_consts["bias_col"]
                    ident_sb = routing_consts["ident_sb"]
                    bmask_sb = routing_consts["bmask_sb"]
                    c0_sb = routing_consts["c0_sb"]

                    pT = ppool.tile([128, N_IT * O], F32R, tag="predT")
                    for t in range(N_IT):
                        ps = ps_conv.tile([128, 512], F32, tag="cv")
                        for k in range(N_KT):
                            nc.tensor.matmul(
                                ps[:, :O],
                                r(x_sb[:, k * I + t * 128:k * I + t * 128 + 128]),
                                r(wt_sb[:, k * O:(k + 1) * O]),
                                start=(k == 0),
                                stop=(k == N_KT - 1),
                            )
                        # eviction fused with the conv-bias add
                        nc.vector.tensor_tensor(
                            pT[:, t * O:(t + 1) * O], ps[:, :O], bias_b128, OP.add
                        )
                    predT.append(pT)

                    pr = ppool.tile([128, N_OT * I], F32R, tag="pred")
                    for m in range(N_OT):
                        mw = 128 if m < 2 else 64
                        for h in range(2):
                            ps = ps_conv.tile([128, 512], F32, tag="cv")
                            for k in range(N_KT):
                                nc.tensor.matmul(
                                    ps[:mw],
                                    r(wt_sb[:, k * O + m * 128:k * O + m * 128 + mw]),
                                    r(x_sb[:, k * I + h * 512:k * I + h * 512 + 512]),
                                    start=(k == 0),
                                    stop=(k == N_KT - 1),
                                )
                            dst_pr = pr[:mw, m * I + h * 512:m * I + h * 512 + 512]
                            if (m * 2 + h) % 2 == 0:
                                nc.scalar.activation(
                                    dst_pr, ps[:mw], AF.Identity,
                                    bias=bias_col[0:mw, m:m + 1], scale=1.0,
                                )
                            else:
                                nc.vector.tensor_scalar_add(
                                    dst_pr, ps[:mw], bias_col[0:mw, m:m + 1]
                                )
                    pred.append(pr)

                # ======== routing ========
                if stage < 2:
                    for b in range(WAVE):
                        gb = wave * WAVE + b
                        dump = state.tile([128, D], F32, tag="v_cmp")
                        nc.vector.tensor_copy(dump[:J], predT[b][:J, :D])
                        nc.vector.tensor_add(dump[:J], dump[:J], pred[b][:J, :D].bitcast(F32))
                        nc.sync.dma_start(out=out_d.ap()[gb], in_=dump[:J])
                    continue
                b_sb = state.tile([128, WAVE * N_IT * J], F32, tag="b")
                c_sb = state.tile([128, WAVE * N_IT * J], F32R, tag="c")
                for it in range(ROUTE_NUM):
                    last = it == ROUTE_NUM - 1
                    V_sb = state.tile([128, N_OT * WAVE * J], F32R, tag="V")
                    for b in range(WAVE):
                        if it > 0:
                            # per-batch softmax over j (free-dim groups of 10)
                            sl = slice(b * N_IT * J, (b + 1) * N_IT * J)
                            e_sb = state.tile([128, N_IT * J], F32, tag="e")
                            nc.scalar.activation(e_sb, b_sb[:, sl], AF.Exp)
                            den = state.tile([128, N_IT], F32, tag="den")
                            nc.vector.reduce_sum(
                                den,
                                e_sb.rearrange("p (g j) -> p g j", j=J),
                                axis=mybir.AxisListType.X,
                            )
                            rden = state.tile([128, N_IT], F32, tag="rden")
                            nc.vector.reciprocal(rden, den)
                            nc.vector.tensor_tensor(
                                c_sb[:, sl].rearrange("p (g j) -> p g j", j=J),
                                e_sb.rearrange("p (g j) -> p g j", j=J),
                                rden.broadcast_to([128, N_IT, J]),
                                OP.mult,
                            )
                        # ---- s = c . predT  (contraction over i) ----
                        ps_s = ps_st.tile([128, O], F32, tag="s")
                        for t in range(N_IT):
                            lhs = (
                                c0_sb
                                if it == 0
                                else c_sb[:, (b * N_IT + t) * J:(b * N_IT + t + 1) * J]
                            )
                            nc.tensor.matmul(
                                ps_s[:J],
                                r(lhs),
                                r(predT[b][:, t * O:(t + 1) * O]),
                                start=(t == 0),
                                stop=(t == N_IT - 1),
                            )

                        # ---- squash (partitions = j) ----
                        s_m = state.tile([128, O], F32, tag="s_m")
                        nc.vector.tensor_tensor(s_m[:J], ps_s[:J], bmask_sb[:J], OP.mult)
                        sq = state.tile([128, O], F32, tag="sq")
                        ns = state.tile([128, 1], F32, tag="ns")
                        nc.scalar.activation(
                            sq[:J], s_m[:J], AF.Square, accum_out=ns[:J]
                        )
                        # sqrt(ns) = exp(0.5*ln(ns)) — keeps every ACT func
                        # in the natural_log_exp_and_others table (one load,
                        # no per-iteration table thrash)
                        lns = state.tile([128, 1], F32, tag="lns")
                        nc.scalar.activation(lns[:J], ns[:J], AF.Ln)
                        rt = state.tile([128, 1], F32, tag="rt")
                        nc.scalar.activation(rt[:J], lns[:J], AF.Exp, scale=0.5)
                        ns1 = state.tile([128, 1], F32, tag="ns1")
                        nc.vector.tensor_scalar_add(ns1[:J], ns[:J], 1.0)
                        rns1 = state.tile([128, 1], F32, tag="rns1")
                        nc.vector.reciprocal(rns1[:J], ns1[:J])
                        coeff = state.tile([128, 1], F32, tag="coeff")
                        nc.vector.tensor_tensor(coeff[:J], rt[:J], rns1[:J], OP.mult)
                        v_full = state.tile([128, O], F32, tag="v_full")
                        nc.vector.tensor_scalar_mul(v_full[:J], s_m[:J], coeff[:J])

                        if last or stage == 2:
                            v_cmp = state.tile([128, D], F32, tag="v_cmp")
                            nc.vector.reduce_sum(
                                v_cmp[:J],
                                v_full[:J].rearrange("p (j d) -> p d j", j=J),
                                axis=mybir.AxisListType.X,
                            )
                            if last or (stage == 2 and it == 0):
                                gb = wave * WAVE + b
                                nc.sync.dma_start(out=out_d.ap()[gb], in_=v_cmp[:J])
                            continue

                        # ---- V: transpose v into [o-part, (k, b, j)] ----
                        ps_tv = ps_tp.tile([128, N_OT * J], F32, tag="T")
                        nc.vector.memset(ps_tv[64:, 2 * J:3 * J], 0.0)
                        for k in range(N_OT):
                            kw = 128 if k < 2 else 64
                            nc.tensor.transpose(
                                ps_tv[:kw, k * J:(k + 1) * J],
                                v_full[:J, k * 128:k * 128 + kw],
                                ident_sb[:J, :J],
                            )
                        vdst = bass.AP(
                            tensor=V_sb.tensor,
                            offset=V_sb.offset + b * J,
                            ap=[list(V_sb.ap[0]), [WAVE * J, N_OT], [1, J]],
                        )
                        nc.vector.tensor_copy(vdst, ps_tv.rearrange("p (k j) -> p k j", j=J))

                    if last or stage == 2:
                        if stage == 2:
                            break
                        continue

                    if stage == 3:
                        for b in range(WAVE):
                            gb = wave * WAVE + b
                            dmp = state.tile([128, D], F32, tag="v_cmp")
                            nc.vector.tensor_copy(dmp[:J], V_sb[:J, :D].bitcast(F32))
                            nc.sync.dma_start(out=out_d.ap()[gb], in_=dmp[:J])
                        break
                    for b in range(WAVE):
                        # ---- delta = V . pred  (contraction over o) ----
                        delta_sb = state.tile([128, I], F32, tag="delta")
                        for h in range(2):
                            ps_d = ps_dp.tile([128, 512], F32, tag="d")
                            for k in range(N_OT):
                                kw = 128 if k < 2 else 64
                                nc.tensor.matmul(
                                    ps_d[:J],
                                    r(V_sb[:kw, (k * WAVE + b) * J:(k * WAVE + b + 1) * J]),
                                    r(pred[b][:kw, k * I + h * 512:k * I + (h + 1) * 512]),
                                    start=(k == 0),
                                    stop=(k == N_OT - 1),
                                )
                            nc.scalar.copy(delta_sb[:J, h * 512:(h + 1) * 512], ps_d[:J])

                        if stage == 4:
                            gb = wave * WAVE + b
                            dmp2 = state.tile([128, D], F32, tag="v_cmp")
                            nc.vector.tensor_copy(dmp2[:J], delta_sb[:J, :D])
                            nc.sync.dma_start(out=out_d.ap()[gb], in_=dmp2[:J])
                            continue
                        # ---- transpose delta back into [i-part, j] ----
                        ps_t = ps_tp.tile([128, N_IT * J], F32, tag="T")
                        for t in range(N_IT):
                            nc.tensor.transpose(
                                ps_t[:, t * J:(t + 1) * J],
                                delta_sb[:J, t * 128:(t + 1) * 128],
                                ident_sb[:J, :J],
                            )
                        dst = b_sb[:, b * N_IT * J:(b + 1) * N_IT * J]
                        if it == 0:
                            nc.vector.tensor_copy(dst, ps_t)
                        else:
                            nc.vector.tensor_tensor(dst, ps_t, dst, OP.add)
                    if stage == 4:
                        break

    nc.compile()
    return nc


_NC_CACHE = None
LAST_RESULT = None


def kernel(x: np.ndarray, W: np.ndarray, W_b: np.ndarray) -> np.ndarray:
    global _NC_CACHE
    if _NC_CACHE is None:
        _NC_CACHE = build_kernel()
    nc = _NC_CACHE

    x = np.ascontiguousarray(x.reshape(BS, C_IN, I), dtype=np.float32)
    wt = np.ascontiguousarray(W.T, dtype=np.float32)
    wb = np.ascontiguousarray(W_b.reshape(1, O), dtype=np.float32)

    in_maps = [
        {
            "x": np.ascontiguousarray(x[c * B_PER_CORE:(c + 1) * B_PER_CORE]),
            "wt": wt,
            "wb": wb,
        }
        for c in range(N_CORES)
    ]
    import os
    trace = bool(int(os.environ.get("KERNEL_TRACE", "0")))
    res = run_bass_kernel_spmd(
        nc, in_maps, core_ids=list(range(N_CORES)), trace=trace
    )
    if trace:
        global LAST_RESULT
        LAST_RESULT = res
    out = np.concatenate([res.results[c]["v"] for c in range(N_CORES)], axis=0)
    return out.astype(np.float32)


if __name__ == "__main__":
    rng = np.random.default_rng(0)
    x = rng.standard_normal((BS, C_IN, 32, 32), dtype=np.float32)
    W = (rng.standard_normal((O, C_IN)) * 0.02).astype(np.float32)
    W_b = (rng.standard_normal((O,)) * 0.02).astype(np.float32)
    v = kernel(x=x, W=W, W_b=W_b)
    print(v.shape, v.dtype, float(np.abs(v).max()))



# revision 9
# speedup vs baseline: 1.0715x; 1.0715x over previous
"""Trainium2 Bass kernel for CapLayer2 (1x1-conv capsule layer with dynamic routing).

Sharding: data-parallel over batch — 8 batches per core on 8 NeuronCores.

Per-core design (single wave of 8 batches, software-pipelined routing):
  - The 1x1 conv produces BOTH pred layouts on TensorE:
      predT [i-part, o] fp32  (for the s matmuls, contraction over i=1024)
      pred  [o-part, i] bf16  (stationary operand of the delta matmuls)
    Conv bias is folded into the evictions (DVE tensor-add against a
    partition-broadcast bias for predT; ACT per-partition bias for pred).
  - delta is computed TRANSPOSED: matmul(lhsT=pred[o,i-chunk], rhs=V[o,j])
    accumulates [i-part, j] directly into a per-batch PSUM b tile across
    BOTH routing iterations (start only on iter 0), so there are no
    [10,1024] delta streams, no delta transposes and no b evictions at all.
  - Routing state b lives in PSUM as [i-part, (itile, j)]; softmax over j
    (J=10) reads PSUM directly and is a free-dim grouped reduction.
  - s/squash per batch use [10, 320] tiles; norms via ACT Square+accum;
    sqrt as exp(0.5*ln) with the ACT table pinned so it never reloads.
  - Engine balance: conv evictions split DVE(predT)/ACT(pred); softmax
    normalize and v_full scaling run on the otherwise-idle GPSIMD (Pool).
  - Final v is reduced to compact [10, 32] per batch and written with a
    single gathered DMA for all 8 batches.
"""

import numpy as np
from contextlib import ExitStack

import concourse.bacc as bacc
import concourse.bass as bass
import concourse.hw_specs as hw_specs

# Force every activation onto the one table that contains all functions this
# kernel uses (Copy/Identity/Exp/Ln/Square) so the ACT engine loads its
# function table exactly once instead of thrashing between sets.
_ONE_TABLE = "natural_log_exp_and_others"
_orig_get_tables = hw_specs.get_activation_tables


def _pinned_tables(arch):
    tabs = _orig_get_tables(arch)
    return {k: (v if k == _ONE_TABLE else set()) for k, v in tabs.items()}


bacc.get_activation_tables = _pinned_tables
import concourse.tile as tile
from concourse import mybir
from concourse.bass_utils import run_bass_kernel_spmd

F32 = mybir.dt.float32
F32R = mybir.dt.float32r
BF16 = mybir.dt.bfloat16
AF = mybir.ActivationFunctionType
OP = mybir.AluOpType
AX = mybir.AxisListType

N_CORES = 8
BS = 64
C_IN = 256
J = 10
D = 32
O = J * D          # 320
I = 1024           # 32*32 pixels
ROUTE_NUM = 3
B = BS // N_CORES  # 8 batches per core
N_IT = I // 128    # 8
N_KT = C_IN // 128 # 2
N_OT = 3           # o tiles: 128, 128, 64
PIPE = 4           # routing software-pipeline depth


def r(ap):
    return ap.bitcast(F32R)


def build_kernel():
    nc = bacc.Bacc("TRN2", target_bir_lowering=False, debug=False, num_devices=1)

    x_d = nc.dram_tensor("x", [B, C_IN, I], F32R, kind="ExternalInput")
    wt_d = nc.dram_tensor("wt", [C_IN, O], F32R, kind="ExternalInput")   # W.T
    wb_d = nc.dram_tensor("wb", [1, O], F32R, kind="ExternalInput")
    out_d = nc.dram_tensor("v", [B, J, D], F32, kind="ExternalOutput")

    ident_d = nc.inline_tensor(np.eye(16, dtype=np.float32), name="ident")
    bm = np.zeros((16, O), dtype=np.float32)
    for j in range(J):
        bm[j, D * j:D * j + D] = 1.0
    bmask_d = nc.inline_tensor(bm, name="bmask")
    c0_d = nc.inline_tensor(np.full((128, J), 1.0 / J, dtype=np.float32), name="c0")

    with tile.TileContext(nc) as tc:
        with ExitStack() as ctx:
            consts = ctx.enter_context(tc.tile_pool(name="consts", bufs=1))
            xpool = ctx.enter_context(tc.tile_pool(name="xp", bufs=4))
            ppT = ctx.enter_context(tc.tile_pool(name="ppT", bufs=B))
            ppO = ctx.enter_context(tc.tile_pool(name="ppO", bufs=B))
            st = ctx.enter_context(tc.tile_pool(name="st", bufs=PIPE))
            cpool = ctx.enter_context(tc.tile_pool(name="cp", bufs=PIPE))
            vpool = ctx.enter_context(tc.tile_pool(name="vp", bufs=PIPE))

            # ---- constants (wt first: it gates the first conv matmul) ----
            wt_sb = consts.tile([128, N_KT * O], F32R)
            nc.sync.dma_start(
                out=wt_sb.rearrange("p (k o) -> p k o", o=O),
                in_=wt_d.ap().rearrange("(k p) o -> p k o", p=128),
            )
            routing_consts = {}

            def load_consts():
                # Emitted after batch 0's x DMA so these small transfers don't
                # delay the startup-critical x.
                bias_b128 = consts.tile([128, O], F32)
                wb_bc = bass.AP(
                    tensor=wb_d, offset=0, ap=[[0, 128], [1, O]]
                ).bitcast(F32)
                nc.sync.dma_start(out=bias_b128, in_=wb_bc)
                bias_col = consts.tile([128, N_OT], F32)
                for m in range(N_OT):
                    mw = 128 if m < 2 else 64
                    nc.sync.dma_start(
                        out=bias_col[0:mw, m:m + 1],
                        in_=wb_d.ap().bitcast(F32)[0:1, 128 * m:128 * m + mw],
                    )
                ident_sb = consts.tile([128, 16], F32)
                nc.sync.dma_start(out=ident_sb[:16], in_=ident_d.ap())
                bmask_sb = consts.tile([128, O], F32)
                nc.sync.dma_start(out=bmask_sb[:16], in_=bmask_d.ap())
                c0_sb = consts.tile([128, J], F32R)
                nc.sync.dma_start(out=c0_sb, in_=r(c0_d.ap()))
                vout = consts.tile([128, B * D], F32)
                routing_consts.update(
                    bias_b128=bias_b128, bias_col=bias_col, ident_sb=ident_sb,
                    bmask_sb=bmask_sb, c0_sb=c0_sb, vout=vout,
                )

            # ======== conv: both layouts, all 8 batches ========
            predT, pred = [], []
            with tc.tile_pool(name="psc", bufs=2, space="PSUM") as ps_conv:
                for b in range(B):
                    x_sb = xpool.tile([128, N_KT * I], F32R, tag="x")
                    for k in range(N_KT):
                        nc.sync.dma_start(
                            out=x_sb[:, k * I:(k + 1) * I],
                            in_=x_d.ap()[b][k * 128:(k + 1) * 128, :],
                        )
                    if b == 0:
                        load_consts()
                    bias_b128 = routing_consts["bias_b128"]
                    bias_col = routing_consts["bias_col"]

                    pT = ppT.tile([128, N_IT * O], F32R, tag="predT")
                    for t in range(N_IT):
                        ps = ps_conv.tile([128, 512], F32, tag="cv")
                        for k in range(N_KT):
                            nc.tensor.matmul(
                                ps[:, :O],
                                r(x_sb[:, k * I + t * 128:k * I + t * 128 + 128]),
                                wt_sb[:, k * O:(k + 1) * O],
                                start=(k == 0),
                                stop=(k == N_KT - 1),
                            )
                        # eviction fused with the conv-bias add
                        nc.vector.tensor_tensor(
                            pT[:, t * O:(t + 1) * O], ps[:, :O], bias_b128, OP.add
                        )
                    predT.append(pT)

                    pr = ppO.tile([128, N_OT * I], BF16, tag="pred")
                    for m in range(N_OT):
                        mw = 128 if m < 2 else 64
                        for h in range(2):
                            ps = ps_conv.tile([128, 512], F32, tag="cv")
                            for k in range(N_KT):
                                nc.tensor.matmul(
                                    ps[:mw],
                                    wt_sb[:, k * O + m * 128:k * O + m * 128 + mw],
                                    r(x_sb[:, k * I + h * 512:k * I + (h + 1) * 512]),
                                    start=(k == 0),
                                    stop=(k == N_KT - 1),
                                )
                            nc.scalar.activation(
                                pr[:mw, m * I + h * 512:m * I + (h + 1) * 512],
                                ps[:mw], AF.Identity,
                                bias=bias_col[0:mw, m:m + 1], scale=1.0,
                            )
                    pred.append(pr)

            # ======== routing ========
            psb = ctx.enter_context(tc.tile_pool(name="psb", bufs=2, space="PSUM"))
            pss = ctx.enter_context(tc.tile_pool(name="pss", bufs=2, space="PSUM"))
            pst = ctx.enter_context(tc.tile_pool(name="pst", bufs=2, space="PSUM"))
            bias_col = routing_consts["bias_col"]
            ident_sb = routing_consts["ident_sb"]
            bmask_sb = routing_consts["bmask_sb"]
            c0_sb = routing_consts["c0_sb"]
            vout = routing_consts["vout"]

            b_sb = consts.tile([128, B * N_IT * J], F32)

            def bslice(b):
                off = b * N_IT * J
                return b_sb[:, off:off + N_IT * J]

            for it in range(ROUTE_NUM):
                last = it == ROUTE_NUM - 1

                def front(b, it=it):
                    """softmax (it>0) + the s matmuls for batch b."""
                    if it > 0:
                        e_sb = st.tile([128, N_IT * J], F32, tag="e")
                        nc.scalar.activation(e_sb, bslice(b), AF.Exp)
                        den = st.tile([128, N_IT], F32, tag="den")
                        nc.vector.reduce_sum(
                            den,
                            e_sb.rearrange("p (g j) -> p g j", j=J),
                            axis=AX.X,
                        )
                        rden = st.tile([128, N_IT], F32, tag="rden")
                        nc.vector.reciprocal(rden, den)
                        c_sb = cpool.tile([128, N_IT * J], F32R, tag="c")
                        nc.vector.tensor_tensor(
                            c_sb.rearrange("p (g j) -> p g j", j=J),
                            e_sb.rearrange("p (g j) -> p g j", j=J),
                            rden.broadcast_to([128, N_IT, J]),
                            OP.mult,
                        )
                    else:
                        c_sb = None
                    ps_s = pss.tile([128, O], F32, tag="s")
                    for t in range(N_IT):
                        lhs = c0_sb if it == 0 else c_sb[:, t * J:(t + 1) * J]
                        nc.tensor.matmul(
                            ps_s[:J],
                            lhs,
                            predT[b][:, t * O:(t + 1) * O],
                            start=(t == 0),
                            stop=(t == N_IT - 1),
                        )
                    return ps_s

                def back(b, ps_s, it=it, last=last):
                    """squash + (V transpose + delta matmuls) | final output."""
                    s_m = st.tile([128, O], F32, tag="s_m")
                    nc.vector.tensor_tensor(s_m[:J], ps_s[:J], bmask_sb[:J], OP.mult)
                    ns = st.tile([128, 1], F32, tag="ns")
                    if not last:
                        sq = st.tile([128, O], F32, tag="sq")
                        nc.scalar.activation(
                            sq[:J], s_m[:J], AF.Square, accum_out=ns[:J]
                        )
                    else:
                        s_cmp = st.tile([128, D], F32, tag="s_cmp")
                        nc.vector.reduce_sum(
                            s_cmp[:J],
                            s_m[:J].rearrange("p (j d) -> p d j", j=J),
                            axis=AX.X,
                        )
                        sq = st.tile([128, D], F32, tag="sqc")
                        nc.scalar.activation(
                            sq[:J], s_cmp[:J], AF.Square, accum_out=ns[:J]
                        )
                    # sqrt(ns) = exp(0.5*ln(ns)) — keeps every ACT func in the
                    # natural_log_exp_and_others table (one load, no thrash)
                    lns = st.tile([128, 1], F32, tag="lns")
                    nc.scalar.activation(lns[:J], ns[:J], AF.Ln)
                    rt = st.tile([128, 1], F32, tag="rt")
                    nc.scalar.activation(rt[:J], lns[:J], AF.Exp, scale=0.5)
                    ns1 = st.tile([128, 1], F32, tag="ns1")
                    nc.vector.tensor_scalar_add(ns1[:J], ns[:J], 1.0)
                    rns1 = st.tile([128, 1], F32, tag="rns1")
                    nc.vector.reciprocal(rns1[:J], ns1[:J])
                    coeff = st.tile([128, 1], F32, tag="coeff")
                    nc.vector.tensor_tensor(coeff[:J], rt[:J], rns1[:J], OP.mult)

                    if last:
                        nc.vector.tensor_scalar_mul(
                            vout[:J, b * D:(b + 1) * D], s_cmp[:J], coeff[:J]
                        )
                        return

                    v_full = st.tile([128, O], F32, tag="v_full")
                    nc.gpsimd.tensor_scalar_mul(v_full[:J], s_m[:J], coeff[:J])

                    # V: transpose v into block-diagonal [o-part, j]
                    ps_tv = pst.tile([128, 32], F32, tag="tv")
                    for k in range(N_OT):
                        kw = 128 if k < 2 else 64
                        nc.tensor.transpose(
                            ps_tv[:kw, k * J:(k + 1) * J],
                            v_full[:J, k * 128:k * 128 + kw],
                            ident_sb[:J, :J],
                        )
                    vb = vpool.tile([128, 32], BF16, tag="vb")
                    nc.vector.tensor_copy(vb[:, :2 * J], ps_tv[:, :2 * J])
                    nc.vector.tensor_copy(vb[:64, 2 * J:3 * J], ps_tv[:64, 2 * J:3 * J])

                    # delta^T: [i-part, j] tiles straight from PE, no transposes
                    d_ps = psb.tile([128, N_IT * J], F32, tag="d")
                    for t in range(N_IT):
                        for k in range(N_OT):
                            kw = 128 if k < 2 else 64
                            nc.tensor.matmul(
                                d_ps[:, t * J:(t + 1) * J],
                                pred[b][:kw, k * I + t * 128:k * I + t * 128 + 128],
                                vb[:kw, k * J:(k + 1) * J],
                                start=(k == 0),
                                stop=(k == N_OT - 1),
                            )
                    dst = bslice(b)
                    if it == 0:
                        nc.vector.tensor_copy(dst, d_ps)
                    else:
                        nc.vector.tensor_tensor(dst, d_ps, dst, OP.add)

                inflight = {}
                for b in range(min(PIPE, B)):
                    inflight[b] = front(b)
                for b in range(B):
                    back(b, inflight.pop(b))
                    nb = b + PIPE
                    if nb < B:
                        inflight[nb] = front(nb)

            # one gathered DMA for all 8 batches
            nc.sync.dma_start(
                out=out_d.ap().rearrange("b j d -> j b d"),
                in_=vout[:J].rearrange("p (b d) -> p b d", d=D),
            )

    nc.compile()
    return nc


_NC_CACHE = None
LAST_RESULT = None


def kernel(x: np.ndarray, W: np.ndarray, W_b: np.ndarray) -> np.ndarray:
    global _NC_CACHE
    if _NC_CACHE is None:
        _NC_CACHE = build_kernel()
    nc = _NC_CACHE

    x = np.ascontiguousarray(x.reshape(BS, C_IN, I), dtype=np.float32)
    wt = np.ascontiguousarray(W.T, dtype=np.float32)
    wb = np.ascontiguousarray(W_b.reshape(1, O), dtype=np.float32)

    in_maps = [
        {
            "x": np.ascontiguousarray(x[c * B:(c + 1) * B]),
            "wt": wt,
            "wb": wb,
        }
        for c in range(N_CORES)
    ]
    import os
    trace = bool(int(os.environ.get("KERNEL_TRACE", "0")))
    res = run_bass_kernel_spmd(
        nc, in_maps, core_ids=list(range(N_CORES)), trace=trace
    )
    if trace:
        global LAST_RESULT
        LAST_RESULT = res
    out = np.concatenate([res.results[c]["v"] for c in range(N_CORES)], axis=0)
    return out.astype(np.float32)


if __name__ == "__main__":
    rng = np.random.default_rng(0)
    x = rng.standard_normal((BS, C_IN, 32, 32), dtype=np.float32)
    W = (rng.standard_normal((O, C_IN)) * 0.02).astype(np.float32)
    W_b = (rng.standard_normal((O,)) * 0.02).astype(np.float32)
    v = kernel(x=x, W=W, W_b=W_b)
    print(v.shape, v.dtype, float(np.abs(v).max()))


# revision 11
# speedup vs baseline: 1.3135x; 1.2259x over previous
"""Trainium2 Bass kernel for CapLayer2 (1x1-conv capsule layer with dynamic routing).

Sharding: data-parallel over batch — 8 batches per core on 8 NeuronCores.

Per-core design (single wave of 8 batches, software-pipelined routing):
  - The 1x1 conv produces BOTH pred layouts on TensorE:
      predT [i-part, o] fp32  (for the s matmuls, contraction over i=1024)
      pred  [o-part, i] bf16  (stationary operand of the delta matmuls)
    Conv bias is folded into the evictions (DVE tensor-add against a
    partition-broadcast bias for predT; ACT per-partition bias for pred).
  - delta is computed TRANSPOSED: matmul(lhsT=pred[o,i-chunk], rhs=V[o,j])
    accumulates [i-part, j] directly into a per-batch PSUM b tile across
    BOTH routing iterations (start only on iter 0), so there are no
    [10,1024] delta streams, no delta transposes and no b evictions at all.
  - Routing state b lives in PSUM as [i-part, (itile, j)]; softmax over j
    (J=10) reads PSUM directly and is a free-dim grouped reduction.
  - s/squash per batch use [10, 320] tiles; norms via ACT Square+accum;
    sqrt as exp(0.5*ln) with the ACT table pinned so it never reloads.
  - Engine balance: conv evictions split DVE(predT)/ACT(pred); softmax
    normalize and v_full scaling run on the otherwise-idle GPSIMD (Pool).
  - Final v is reduced to compact [10, 32] per batch and written with a
    single gathered DMA for all 8 batches.
"""

import numpy as np
from contextlib import ExitStack

import concourse.bacc as bacc
import concourse.bass as bass
import concourse.hw_specs as hw_specs

# Force every activation onto the one table that contains all functions this
# kernel uses (Copy/Identity/Exp/Ln/Square) so the ACT engine loads its
# function table exactly once instead of thrashing between sets.
_ONE_TABLE = "natural_log_exp_and_others"
_orig_get_tables = hw_specs.get_activation_tables


def _pinned_tables(arch):
    tabs = _orig_get_tables(arch)
    return {k: (v if k == _ONE_TABLE else set()) for k, v in tabs.items()}


bacc.get_activation_tables = _pinned_tables
import concourse.tile as tile
from concourse import mybir
from concourse.bass_utils import run_bass_kernel_spmd

F32 = mybir.dt.float32
F32R = mybir.dt.float32r
BF16 = mybir.dt.bfloat16
AF = mybir.ActivationFunctionType
OP = mybir.AluOpType
AX = mybir.AxisListType

N_CORES = 8
BS = 64
C_IN = 256
J = 10
D = 32
O = J * D          # 320
I = 1024           # 32*32 pixels
ROUTE_NUM = 3
B = BS // N_CORES  # 8 batches per core
N_IT = I // 128    # 8
N_KT = C_IN // 128 # 2
N_OT = 3           # o tiles: 128, 128, 64
PIPE = 4           # routing software-pipeline depth


def r(ap):
    return ap.bitcast(F32R)


def build_kernel():
    nc = bacc.Bacc("TRN2", target_bir_lowering=False, debug=False, num_devices=1)

    x_d = nc.dram_tensor("x", [B, C_IN, I], F32R, kind="ExternalInput")
    wt_d = nc.dram_tensor("wt", [C_IN, O], F32R, kind="ExternalInput")   # W.T
    wb_d = nc.dram_tensor("wb", [1, O], F32R, kind="ExternalInput")
    out_d = nc.dram_tensor("v", [B, J, D], F32, kind="ExternalOutput")

    ident_d = nc.inline_tensor(np.eye(16, dtype=np.float32), name="ident")
    bm = np.zeros((16, O), dtype=np.float32)
    for j in range(J):
        bm[j, D * j:D * j + D] = 1.0
    bmask_d = nc.inline_tensor(bm, name="bmask")
    c0_d = nc.inline_tensor(np.full((128, J), 1.0 / J, dtype=np.float32), name="c0")

    with tile.TileContext(nc) as tc:
        with ExitStack() as ctx:
            consts = ctx.enter_context(tc.tile_pool(name="consts", bufs=1))
            xpool = ctx.enter_context(tc.tile_pool(name="xp", bufs=4))
            ppT = ctx.enter_context(tc.tile_pool(name="ppT", bufs=B))
            ppO = ctx.enter_context(tc.tile_pool(name="ppO", bufs=B))
            st = ctx.enter_context(tc.tile_pool(name="st", bufs=PIPE))
            cpool = ctx.enter_context(tc.tile_pool(name="cp", bufs=PIPE))
            vpool = ctx.enter_context(tc.tile_pool(name="vp", bufs=PIPE))

            # ---- constants (wt first: it gates the first conv matmul) ----
            wt_sb = consts.tile([128, N_KT * O], F32R)
            nc.sync.dma_start(
                out=wt_sb.rearrange("p (k o) -> p k o", o=O),
                in_=wt_d.ap().rearrange("(k p) o -> p k o", p=128),
            )
            routing_consts = {}

            def load_consts():
                # Emitted after batch 0's x DMA so these small transfers don't
                # delay the startup-critical x.
                bias_b128 = consts.tile([128, O], F32)
                wb_bc = bass.AP(
                    tensor=wb_d, offset=0, ap=[[0, 128], [1, O]]
                ).bitcast(F32)
                nc.sync.dma_start(out=bias_b128, in_=wb_bc)
                bias_col = consts.tile([128, N_OT], F32)
                for m in range(N_OT):
                    mw = 128 if m < 2 else 64
                    nc.sync.dma_start(
                        out=bias_col[0:mw, m:m + 1],
                        in_=wb_d.ap().bitcast(F32)[0:1, 128 * m:128 * m + mw],
                    )
                ident_sb = consts.tile([128, 16], F32)
                nc.sync.dma_start(out=ident_sb[:16], in_=ident_d.ap())
                bmask_sb = consts.tile([128, O], F32)
                nc.sync.dma_start(out=bmask_sb[:16], in_=bmask_d.ap())
                c0_sb = consts.tile([128, J], F32R)
                nc.sync.dma_start(out=c0_sb, in_=r(c0_d.ap()))
                vout = consts.tile([128, B * D], F32)
                routing_consts.update(
                    bias_b128=bias_b128, bias_col=bias_col, ident_sb=ident_sb,
                    bmask_sb=bmask_sb, c0_sb=c0_sb, vout=vout,
                )

            # ======== conv: both layouts, all 8 batches ========
            predT, pred = [], []
            with tc.tile_pool(name="psc", bufs=4, space="PSUM") as ps_conv:
                for b in range(B):
                    x_sb = xpool.tile([128, N_KT * I], F32R, tag="x")
                    for k in range(N_KT):
                        nc.sync.dma_start(
                            out=x_sb[:, k * I:(k + 1) * I],
                            in_=x_d.ap()[b][k * 128:(k + 1) * 128, :],
                        )
                    if b == 0:
                        load_consts()
                    bias_b128 = routing_consts["bias_b128"]
                    bias_col = routing_consts["bias_col"]

                    pT = ppT.tile([128, N_IT * O], F32R, tag="predT")
                    pr = ppO.tile([128, N_OT * I], BF16, tag="pred")
                    # Interleave the two layouts so the DVE (predT) and ACT
                    # (pred) evictions run concurrently instead of in phases.
                    jobs = []
                    for u in range(N_IT):
                        jobs.append(("T", u))
                        if u < 2 * N_OT:
                            jobs.append(("O", u))
                    for kind, u in jobs:
                        if kind == "T":
                            t = u
                            ps = ps_conv.tile([128, 512], F32, tag="cv")
                            for k in range(N_KT):
                                nc.tensor.matmul(
                                    ps[:, :O],
                                    r(x_sb[:, k * I + t * 128:k * I + t * 128 + 128]),
                                    wt_sb[:, k * O:(k + 1) * O],
                                    start=(k == 0),
                                    stop=(k == N_KT - 1),
                                )
                            # eviction fused with the conv-bias add
                            nc.vector.tensor_tensor(
                                pT[:, t * O:(t + 1) * O], ps[:, :O], bias_b128, OP.add
                            )
                        else:
                            m, h = divmod(u, 2)
                            mw = 128 if m < 2 else 64
                            ps = ps_conv.tile([128, 512], F32, tag="cv")
                            for k in range(N_KT):
                                nc.tensor.matmul(
                                    ps[:mw],
                                    wt_sb[:, k * O + m * 128:k * O + m * 128 + mw],
                                    r(x_sb[:, k * I + h * 512:k * I + (h + 1) * 512]),
                                    start=(k == 0),
                                    stop=(k == N_KT - 1),
                                )
                            nc.scalar.activation(
                                pr[:mw, m * I + h * 512:m * I + (h + 1) * 512],
                                ps[:mw], AF.Identity,
                                bias=bias_col[0:mw, m:m + 1], scale=1.0,
                            )
                    predT.append(pT)
                    pred.append(pr)

            # ======== routing ========
            psb = ctx.enter_context(tc.tile_pool(name="psb", bufs=2, space="PSUM"))
            pss = ctx.enter_context(tc.tile_pool(name="pss", bufs=3, space="PSUM"))
            pst = ctx.enter_context(tc.tile_pool(name="pst", bufs=2, space="PSUM"))
            bias_col = routing_consts["bias_col"]
            ident_sb = routing_consts["ident_sb"]
            bmask_sb = routing_consts["bmask_sb"]
            c0_sb = routing_consts["c0_sb"]
            vout = routing_consts["vout"]

            b_sb = consts.tile([128, B * N_IT * J], F32)

            def bslice(b):
                off = b * N_IT * J
                return b_sb[:, off:off + N_IT * J]

            for it in range(ROUTE_NUM):
                last = it == ROUTE_NUM - 1

                def front(b, it=it):
                    """softmax (it>0) + the s matmuls for batch b."""
                    if it > 0:
                        e_sb = st.tile([128, N_IT * J], F32, tag="e")
                        nc.scalar.activation(e_sb, bslice(b), AF.Exp)
                        den = st.tile([128, N_IT], F32, tag="den")
                        nc.vector.reduce_sum(
                            den,
                            e_sb.rearrange("p (g j) -> p g j", j=J),
                            axis=AX.X,
                        )
                        rden = st.tile([128, N_IT], F32, tag="rden")
                        nc.vector.reciprocal(rden, den)
                        c_sb = cpool.tile([128, N_IT * J], F32R, tag="c")
                        nc.vector.tensor_tensor(
                            c_sb.rearrange("p (g j) -> p g j", j=J),
                            e_sb.rearrange("p (g j) -> p g j", j=J),
                            rden.broadcast_to([128, N_IT, J]),
                            OP.mult,
                        )
                    else:
                        c_sb = None
                    ps_s = pss.tile([128, O], F32, tag="s")
                    for t in range(N_IT):
                        lhs = c0_sb if it == 0 else c_sb[:, t * J:(t + 1) * J]
                        nc.tensor.matmul(
                            ps_s[:J],
                            lhs,
                            predT[b][:, t * O:(t + 1) * O],
                            start=(t == 0),
                            stop=(t == N_IT - 1),
                        )
                    return ps_s

                def back(b, ps_s, it=it, last=last):
                    """squash + (V transpose + delta matmuls) | final output."""
                    s_m = st.tile([128, O], F32, tag="s_m")
                    nc.vector.tensor_tensor(s_m[:J], ps_s[:J], bmask_sb[:J], OP.mult)
                    ns = st.tile([128, 1], F32, tag="ns")
                    if not last:
                        sq = st.tile([128, O], F32, tag="sq")
                        nc.scalar.activation(
                            sq[:J], s_m[:J], AF.Square, accum_out=ns[:J]
                        )
                    else:
                        s_cmp = st.tile([128, D], F32, tag="s_cmp")
                        nc.vector.reduce_sum(
                            s_cmp[:J],
                            s_m[:J].rearrange("p (j d) -> p d j", j=J),
                            axis=AX.X,
                        )
                        sq = st.tile([128, D], F32, tag="sqc")
                        nc.scalar.activation(
                            sq[:J], s_cmp[:J], AF.Square, accum_out=ns[:J]
                        )
                    # sqrt(ns) = exp(0.5*ln(ns)) — keeps every ACT func in the
                    # natural_log_exp_and_others table (one load, no thrash)
                    lns = st.tile([128, 1], F32, tag="lns")
                    nc.scalar.activation(lns[:J], ns[:J], AF.Ln)
                    rt = st.tile([128, 1], F32, tag="rt")
                    nc.scalar.activation(rt[:J], lns[:J], AF.Exp, scale=0.5)
                    ns1 = st.tile([128, 1], F32, tag="ns1")
                    nc.vector.tensor_scalar_add(ns1[:J], ns[:J], 1.0)
                    rns1 = st.tile([128, 1], F32, tag="rns1")
                    nc.vector.reciprocal(rns1[:J], ns1[:J])
                    coeff = st.tile([128, 1], F32, tag="coeff")
                    nc.vector.tensor_tensor(coeff[:J], rt[:J], rns1[:J], OP.mult)

                    if last:
                        nc.vector.tensor_scalar_mul(
                            vout[:J, b * D:(b + 1) * D], s_cmp[:J], coeff[:J]
                        )
                        return

                    v_full = st.tile([128, O], F32, tag="v_full")
                    nc.gpsimd.tensor_scalar_mul(v_full[:J], s_m[:J], coeff[:J])

                    # V: transpose v into block-diagonal [o-part, j]
                    ps_tv = pst.tile([128, 32], F32, tag="tv")
                    for k in range(N_OT):
                        kw = 128 if k < 2 else 64
                        nc.tensor.transpose(
                            ps_tv[:kw, k * J:(k + 1) * J],
                            v_full[:J, k * 128:k * 128 + kw],
                            ident_sb[:J, :J],
                        )
                    vb = vpool.tile([128, 32], BF16, tag="vb")
                    nc.vector.tensor_copy(vb[:, :2 * J], ps_tv[:, :2 * J])
                    nc.vector.tensor_copy(vb[:64, 2 * J:3 * J], ps_tv[:64, 2 * J:3 * J])

                    # delta^T: [i-part, j] tiles straight from PE, no transposes
                    d_ps = psb.tile([128, N_IT * J], F32, tag="d")
                    for t in range(N_IT):
                        for k in range(N_OT):
                            kw = 128 if k < 2 else 64
                            nc.tensor.matmul(
                                d_ps[:, t * J:(t + 1) * J],
                                pred[b][:kw, k * I + t * 128:k * I + t * 128 + 128],
                                vb[:kw, k * J:(k + 1) * J],
                                start=(k == 0),
                                stop=(k == N_OT - 1),
                            )
                    dst = bslice(b)
                    if it == 0:
                        nc.vector.tensor_copy(dst, d_ps)
                    else:
                        nc.vector.tensor_tensor(dst, d_ps, dst, OP.add)

                inflight = {}
                for b in range(min(PIPE, B)):
                    inflight[b] = front(b)
                for b in range(B):
                    back(b, inflight.pop(b))
                    nb = b + PIPE
                    if nb < B:
                        inflight[nb] = front(nb)

            # one gathered DMA for all 8 batches
            nc.sync.dma_start(
                out=out_d.ap().rearrange("b j d -> j b d"),
                in_=vout[:J].rearrange("p (b d) -> p b d", d=D),
            )

    nc.compile()
    return nc


_NC_CACHE = None
LAST_RESULT = None


def kernel(x: np.ndarray, W: np.ndarray, W_b: np.ndarray) -> np.ndarray:
    global _NC_CACHE
    if _NC_CACHE is None:
        _NC_CACHE = build_kernel()
    nc = _NC_CACHE

    x = np.ascontiguousarray(x.reshape(BS, C_IN, I), dtype=np.float32)
    wt = np.ascontiguousarray(W.T, dtype=np.float32)
    wb = np.ascontiguousarray(W_b.reshape(1, O), dtype=np.float32)

    in_maps = [
        {
            "x": np.ascontiguousarray(x[c * B:(c + 1) * B]),
            "wt": wt,
            "wb": wb,
        }
        for c in range(N_CORES)
    ]
    import os
    trace = bool(int(os.environ.get("KERNEL_TRACE", "0")))
    res = run_bass_kernel_spmd(
        nc, in_maps, core_ids=list(range(N_CORES)), trace=trace
    )
    if trace:
        global LAST_RESULT
        LAST_RESULT = res
    out = np.concatenate([res.results[c]["v"] for c in range(N_CORES)], axis=0)
    return out.astype(np.float32)


if __name__ == "__main__":
    rng = np.random.default_rng(0)
    x = rng.standard_normal((BS, C_IN, 32, 32), dtype=np.float32)
    W = (rng.standard_normal((O, C_IN)) * 0.02).astype(np.float32)
    W_b = (rng.standard_normal((O,)) * 0.02).astype(np.float32)
    v = kernel(x=x, W=W, W_b=W_b)
    print(v.shape, v.dtype, float(np.abs(v).max()))


# revision 12
# speedup vs baseline: 1.3313x; 1.0135x over previous
"""Trainium2 Bass kernel for CapLayer2 (1x1-conv capsule layer with dynamic routing).

Sharding: data-parallel over batch — 8 batches per core on 8 NeuronCores.

Per-core design (single wave of 8 batches, software-pipelined routing):
  - The 1x1 conv produces BOTH pred layouts on TensorE:
      predT [i-part, o] fp32  (for the s matmuls, contraction over i=1024)
      pred  [o-part, i] bf16  (stationary operand of the delta matmuls)
    Conv bias is folded into the evictions (DVE tensor-add against a
    partition-broadcast bias for predT; ACT per-partition bias for pred).
  - delta is computed TRANSPOSED: matmul(lhsT=pred[o,i-chunk], rhs=V[o,j])
    accumulates [i-part, j] directly into a per-batch PSUM b tile across
    BOTH routing iterations (start only on iter 0), so there are no
    [10,1024] delta streams, no delta transposes and no b evictions at all.
  - Routing state b lives in PSUM as [i-part, (itile, j)]; softmax over j
    (J=10) reads PSUM directly and is a free-dim grouped reduction.
  - s/squash per batch use [10, 320] tiles; norms via ACT Square+accum;
    sqrt as exp(0.5*ln) with the ACT table pinned so it never reloads.
  - Engine balance: conv evictions split DVE(predT)/ACT(pred); softmax
    normalize and v_full scaling run on the otherwise-idle GPSIMD (Pool).
  - Final v is reduced to compact [10, 32] per batch and written with a
    single gathered DMA for all 8 batches.
"""

import numpy as np
from contextlib import ExitStack

import concourse.bacc as bacc
import concourse.bass as bass
import concourse.hw_specs as hw_specs

# Force every activation onto the one table that contains all functions this
# kernel uses (Copy/Identity/Exp/Ln/Square) so the ACT engine loads its
# function table exactly once instead of thrashing between sets.
_ONE_TABLE = "natural_log_exp_and_others"
_orig_get_tables = hw_specs.get_activation_tables


def _pinned_tables(arch):
    tabs = _orig_get_tables(arch)
    return {k: (v if k == _ONE_TABLE else set()) for k, v in tabs.items()}


bacc.get_activation_tables = _pinned_tables
import concourse.tile as tile
from concourse import mybir
from concourse.bass_utils import run_bass_kernel_spmd

F32 = mybir.dt.float32
F32R = mybir.dt.float32r
BF16 = mybir.dt.bfloat16
AF = mybir.ActivationFunctionType
OP = mybir.AluOpType
AX = mybir.AxisListType

N_CORES = 8
BS = 64
C_IN = 256
J = 10
D = 32
O = J * D          # 320
I = 1024           # 32*32 pixels
ROUTE_NUM = 3
B = BS // N_CORES  # 8 batches per core
N_IT = I // 128    # 8
N_KT = C_IN // 128 # 2
N_OT = 3           # o tiles: 128, 128, 64
PIPE = 6           # routing software-pipeline depth


def r(ap):
    return ap.bitcast(F32R)


def build_kernel():
    nc = bacc.Bacc("TRN2", target_bir_lowering=False, debug=False, num_devices=1)

    x_d = nc.dram_tensor("x", [B, C_IN, I], F32R, kind="ExternalInput")
    wt_d = nc.dram_tensor("wt", [C_IN, O], F32R, kind="ExternalInput")   # W.T
    wb_d = nc.dram_tensor("wb", [1, O], F32R, kind="ExternalInput")
    out_d = nc.dram_tensor("v", [B, J, D], F32, kind="ExternalOutput")

    ident_d = nc.inline_tensor(np.eye(16, dtype=np.float32), name="ident")
    bm = np.zeros((16, O), dtype=np.float32)
    for j in range(J):
        bm[j, D * j:D * j + D] = 1.0
    bmask_d = nc.inline_tensor(bm, name="bmask")
    c0_d = nc.inline_tensor(np.full((128, J), 1.0 / J, dtype=np.float32), name="c0")

    with tile.TileContext(nc) as tc:
        with ExitStack() as ctx:
            consts = ctx.enter_context(tc.tile_pool(name="consts", bufs=1))
            xpool = ctx.enter_context(tc.tile_pool(name="xp", bufs=4))
            ppT = ctx.enter_context(tc.tile_pool(name="ppT", bufs=B))
            ppO = ctx.enter_context(tc.tile_pool(name="ppO", bufs=B))
            st = ctx.enter_context(tc.tile_pool(name="st", bufs=PIPE))
            cpool = ctx.enter_context(tc.tile_pool(name="cp", bufs=PIPE))
            vpool = ctx.enter_context(tc.tile_pool(name="vp", bufs=PIPE))

            # ---- constants (wt first: it gates the first conv matmul) ----
            wt_sb = consts.tile([128, N_KT * O], F32R)
            nc.sync.dma_start(
                out=wt_sb.rearrange("p (k o) -> p k o", o=O),
                in_=wt_d.ap().rearrange("(k p) o -> p k o", p=128),
            )
            routing_consts = {}

            def load_consts():
                # Emitted after batch 0's x DMA so these small transfers don't
                # delay the startup-critical x.
                bias_b128 = consts.tile([128, O], F32)
                wb_bc = bass.AP(
                    tensor=wb_d, offset=0, ap=[[0, 128], [1, O]]
                ).bitcast(F32)
                nc.sync.dma_start(out=bias_b128, in_=wb_bc)
                bias_col = consts.tile([128, N_OT], F32)
                for m in range(N_OT):
                    mw = 128 if m < 2 else 64
                    nc.sync.dma_start(
                        out=bias_col[0:mw, m:m + 1],
                        in_=wb_d.ap().bitcast(F32)[0:1, 128 * m:128 * m + mw],
                    )
                ident_sb = consts.tile([128, 16], F32)
                nc.sync.dma_start(out=ident_sb[:16], in_=ident_d.ap())
                bmask_sb = consts.tile([128, O], F32)
                nc.sync.dma_start(out=bmask_sb[:16], in_=bmask_d.ap())
                c0_sb = consts.tile([128, J], F32R)
                nc.sync.dma_start(out=c0_sb, in_=r(c0_d.ap()))
                vout = consts.tile([128, B * D], F32)
                routing_consts.update(
                    bias_b128=bias_b128, bias_col=bias_col, ident_sb=ident_sb,
                    bmask_sb=bmask_sb, c0_sb=c0_sb, vout=vout,
                )

            # ======== conv: both layouts, all 8 batches ========
            predT, pred = [], []
            with tc.tile_pool(name="psc", bufs=4, space="PSUM") as ps_conv:
                for b in range(B):
                    x_sb = xpool.tile([128, N_KT * I], F32R, tag="x")
                    for h in range(2):
                        for k in range(N_KT):
                            nc.sync.dma_start(
                                out=x_sb[:, k * I + h * 512:k * I + (h + 1) * 512],
                                in_=x_d.ap()[b][k * 128:(k + 1) * 128, h * 512:(h + 1) * 512],
                            )
                    if b == 0:
                        load_consts()
                    bias_b128 = routing_consts["bias_b128"]
                    bias_col = routing_consts["bias_col"]

                    pT = ppT.tile([128, N_IT * O], F32R, tag="predT")
                    pr = ppO.tile([128, N_OT * I], BF16, tag="pred")
                    # Interleave the two layouts so the DVE (predT) and ACT
                    # (pred) evictions run concurrently instead of in phases.
                    jobs = []
                    for u in range(N_IT):
                        jobs.append(("T", u))
                        if u < 2 * N_OT:
                            jobs.append(("O", u))
                    for kind, u in jobs:
                        if kind == "T":
                            t = u
                            ps = ps_conv.tile([128, 512], F32, tag="cv")
                            for k in range(N_KT):
                                nc.tensor.matmul(
                                    ps[:, :O],
                                    r(x_sb[:, k * I + t * 128:k * I + t * 128 + 128]),
                                    wt_sb[:, k * O:(k + 1) * O],
                                    start=(k == 0),
                                    stop=(k == N_KT - 1),
                                )
                            # eviction fused with the conv-bias add
                            nc.vector.tensor_tensor(
                                pT[:, t * O:(t + 1) * O], ps[:, :O], bias_b128, OP.add
                            )
                        else:
                            m, h = divmod(u, 2)
                            mw = 128 if m < 2 else 64
                            ps = ps_conv.tile([128, 512], F32, tag="cv")
                            for k in range(N_KT):
                                nc.tensor.matmul(
                                    ps[:mw],
                                    wt_sb[:, k * O + m * 128:k * O + m * 128 + mw],
                                    r(x_sb[:, k * I + h * 512:k * I + (h + 1) * 512]),
                                    start=(k == 0),
                                    stop=(k == N_KT - 1),
                                )
                            nc.scalar.activation(
                                pr[:mw, m * I + h * 512:m * I + (h + 1) * 512],
                                ps[:mw], AF.Identity,
                                bias=bias_col[0:mw, m:m + 1], scale=1.0,
                            )
                    predT.append(pT)
                    pred.append(pr)

            # ======== routing ========
            psb = ctx.enter_context(tc.tile_pool(name="psb", bufs=3, space="PSUM"))
            pss = ctx.enter_context(tc.tile_pool(name="pss", bufs=3, space="PSUM"))
            pst = ctx.enter_context(tc.tile_pool(name="pst", bufs=2, space="PSUM"))
            bias_col = routing_consts["bias_col"]
            ident_sb = routing_consts["ident_sb"]
            bmask_sb = routing_consts["bmask_sb"]
            c0_sb = routing_consts["c0_sb"]
            vout = routing_consts["vout"]

            b_sb = consts.tile([128, B * N_IT * J], F32)

            def bslice(b):
                off = b * N_IT * J
                return b_sb[:, off:off + N_IT * J]

            for it in range(ROUTE_NUM):
                last = it == ROUTE_NUM - 1

                def front(b, it=it):
                    """softmax (it>0) + the s matmuls for batch b."""
                    if it > 0:
                        e_sb = st.tile([128, N_IT * J], F32, tag="e")
                        nc.scalar.activation(e_sb, bslice(b), AF.Exp)
                        den = st.tile([128, N_IT], F32, tag="den")
                        nc.vector.reduce_sum(
                            den,
                            e_sb.rearrange("p (g j) -> p g j", j=J),
                            axis=AX.X,
                        )
                        rden = st.tile([128, N_IT], F32, tag="rden")
                        nc.vector.reciprocal(rden, den)
                        c_sb = cpool.tile([128, N_IT * J], F32R, tag="c")
                        nc.vector.tensor_tensor(
                            c_sb.rearrange("p (g j) -> p g j", j=J),
                            e_sb.rearrange("p (g j) -> p g j", j=J),
                            rden.broadcast_to([128, N_IT, J]),
                            OP.mult,
                        )
                    else:
                        c_sb = None
                    ps_s = pss.tile([128, O], F32, tag="s")
                    for t in range(N_IT):
                        lhs = c0_sb if it == 0 else c_sb[:, t * J:(t + 1) * J]
                        nc.tensor.matmul(
                            ps_s[:J],
                            lhs,
                            predT[b][:, t * O:(t + 1) * O],
                            start=(t == 0),
                            stop=(t == N_IT - 1),
                        )
                    return ps_s

                def back(b, ps_s, it=it, last=last):
                    """squash + (V transpose + delta matmuls) | final output."""
                    s_m = st.tile([128, O], F32, tag="s_m")
                    nc.vector.tensor_tensor(s_m[:J], ps_s[:J], bmask_sb[:J], OP.mult)
                    ns = st.tile([128, 1], F32, tag="ns")
                    if not last:
                        sq = st.tile([128, O], F32, tag="sq")
                        nc.scalar.activation(
                            sq[:J], s_m[:J], AF.Square, accum_out=ns[:J]
                        )
                    else:
                        s_cmp = st.tile([128, D], F32, tag="s_cmp")
                        nc.vector.reduce_sum(
                            s_cmp[:J],
                            s_m[:J].rearrange("p (j d) -> p d j", j=J),
                            axis=AX.X,
                        )
                        sq = st.tile([128, D], F32, tag="sqc")
                        nc.scalar.activation(
                            sq[:J], s_cmp[:J], AF.Square, accum_out=ns[:J]
                        )
                    # sqrt(ns) = exp(0.5*ln(ns)) — keeps every ACT func in the
                    # natural_log_exp_and_others table (one load, no thrash)
                    lns = st.tile([128, 1], F32, tag="lns")
                    nc.scalar.activation(lns[:J], ns[:J], AF.Ln)
                    rt = st.tile([128, 1], F32, tag="rt")
                    nc.scalar.activation(rt[:J], lns[:J], AF.Exp, scale=0.5)
                    ns1 = st.tile([128, 1], F32, tag="ns1")
                    nc.gpsimd.tensor_scalar_add(ns1[:J], ns[:J], 1.0)
                    rns1 = st.tile([128, 1], F32, tag="rns1")
                    nc.vector.reciprocal(rns1[:J], ns1[:J])
                    coeff = st.tile([128, 1], F32, tag="coeff")
                    nc.gpsimd.tensor_tensor(coeff[:J], rt[:J], rns1[:J], OP.mult)

                    if last:
                        nc.gpsimd.tensor_scalar_mul(
                            vout[:J, b * D:(b + 1) * D], s_cmp[:J], coeff[:J]
                        )
                        return

                    v_full = st.tile([128, O], F32, tag="v_full")
                    nc.gpsimd.tensor_scalar_mul(v_full[:J], s_m[:J], coeff[:J])

                    # V: transpose v into block-diagonal [o-part, j]
                    ps_tv = pst.tile([128, 32], F32, tag="tv")
                    for k in range(N_OT):
                        kw = 128 if k < 2 else 64
                        nc.tensor.transpose(
                            ps_tv[:kw, k * J:(k + 1) * J],
                            v_full[:J, k * 128:k * 128 + kw],
                            ident_sb[:J, :J],
                        )
                    vb = vpool.tile([128, 32], BF16, tag="vb")
                    nc.scalar.activation(vb[:, :2 * J], ps_tv[:, :2 * J], AF.Identity, scale=1.0)
                    nc.vector.tensor_copy(vb[:64, 2 * J:3 * J], ps_tv[:64, 2 * J:3 * J])

                    # delta^T: [i-part, j] tiles straight from PE, no transposes
                    d_ps = psb.tile([128, N_IT * J], F32, tag="d")
                    for t in range(N_IT):
                        for k in range(N_OT):
                            kw = 128 if k < 2 else 64
                            nc.tensor.matmul(
                                d_ps[:, t * J:(t + 1) * J],
                                pred[b][:kw, k * I + t * 128:k * I + t * 128 + 128],
                                vb[:kw, k * J:(k + 1) * J],
                                start=(k == 0),
                                stop=(k == N_OT - 1),
                            )
                    dst = bslice(b)
                    if it == 0:
                        nc.vector.tensor_copy(dst, d_ps)
                    else:
                        nc.vector.tensor_tensor(dst, d_ps, dst, OP.add)

                inflight = {}
                for b in range(min(PIPE, B)):
                    inflight[b] = front(b)
                for b in range(B):
                    back(b, inflight.pop(b))
                    nb = b + PIPE
                    if nb < B:
                        inflight[nb] = front(nb)

            # one gathered DMA for all 8 batches
            nc.sync.dma_start(
                out=out_d.ap().rearrange("b j d -> j b d"),
                in_=vout[:J].rearrange("p (b d) -> p b d", d=D),
            )

    nc.compile()
    return nc


_NC_CACHE = None
LAST_RESULT = None


def kernel(x: np.ndarray, W: np.ndarray, W_b: np.ndarray) -> np.ndarray:
    global _NC_CACHE
    if _NC_CACHE is None:
        _NC_CACHE = build_kernel()
    nc = _NC_CACHE

    x = np.ascontiguousarray(x.reshape(BS, C_IN, I), dtype=np.float32)
    wt = np.ascontiguousarray(W.T, dtype=np.float32)
    wb = np.ascontiguousarray(W_b.reshape(1, O), dtype=np.float32)

    in_maps = [
        {
            "x": np.ascontiguousarray(x[c * B:(c + 1) * B]),
            "wt": wt,
            "wb": wb,
        }
        for c in range(N_CORES)
    ]
    import os
    trace = bool(int(os.environ.get("KERNEL_TRACE", "0")))
    res = run_bass_kernel_spmd(
        nc, in_maps, core_ids=list(range(N_CORES)), trace=trace
    )
    if trace:
        global LAST_RESULT
        LAST_RESULT = res
    out = np.concatenate([res.results[c]["v"] for c in range(N_CORES)], axis=0)
    return out.astype(np.float32)


if __name__ == "__main__":
    rng = np.random.default_rng(0)
    x = rng.standard_normal((BS, C_IN, 32, 32), dtype=np.float32)
    W = (rng.standard_normal((O, C_IN)) * 0.02).astype(np.float32)
    W_b = (rng.standard_normal((O,)) * 0.02).astype(np.float32)
    v = kernel(x=x, W=W, W_b=W_b)
    print(v.shape, v.dtype, float(np.abs(v).max()))
